# revision 30
# baseline (speedup 1.0000x reference)
"""MultiLevelAlignedRoIPooling Trainium2 kernel.

Strategy
--------
Output[b, n, i, j, c] = sum_{a,b' in {0,1}} w_ab' * feat_lvl[b, y_a(i), x_b'(j), c]
(7x7 aligned bilinear RoI pooling; the 2x2 "avg pool" in the reference is
algebraically the 4-tap bilinear interpolation at each of the 7x7 sample
points).

With the reference's box distribution (h, w in [32, 400] => area_sqrt in
[32, 400] c [32, 448)), *every* box is assigned pyramid level 4, i.e. all
gathers read feat0 only.  We verify this on the host and fall back to a
plain numpy replica of the reference in the (impossible) general case.

Sharding: 8 cores = 4 batches x 2 halves of the 256 boxes.  Each core:
  - The host packs feat0[b] into a "row-pair" table (fp16): row (y,x) holds
    [feat[y,x,:], feat[y+1,x,:]], so ONE 2KB dma_gather element fetches the
    full 2x2 bilinear block of a sample point (both rows x both columns).
  - dma_gather (gpsimd SWDGE) fetches one element per sample point, landing
    on partitions (partition = box, free slot = sample), in j-major order
    (7 chunks = the 7 sample columns).
  - Combine: per chunk, ONE wide ScalarE mul + ONE DVE scalar_tensor_tensor
    does the x-interpolation for all 7 rows at once (x-weights depend only
    on j); then 7 narrow op-pairs do the y-interpolation (y-weights depend
    only on i).  All weights are exact fp32 per-partition scalars.
  - Results stream back to DRAM as [box, 49*256] fp16 rows.

Host prep computes gather indices (int16) + tap weights (f32) with numpy
f32 math that mirrors the reference op-for-op.
"""

import os

import numpy as np

# Set KERNEL_FP32=1 to run the gather/combine pipeline in float32 instead
# of float16 (slower, slightly more accurate).
FP32 = os.environ.get("KERNEL_FP32", "0") == "1"
FDT = np.float32 if FP32 else np.float16
# gather implementation: dma_gather (default, proven) vs indirect_dma_start
# (experimental: crashed the NRT exec unit on TRN2 — do not enable).
INDIRECT = os.environ.get("KERNEL_INDIRECT", "0") == "1"

B, N, C = 4, 256, 256
H = W = 128
OUT = 7
NS = OUT * OUT            # 49 sample points per box
BOX_PER_CORE = 128
NCORES = 8
# j-major sample order: chunk j holds the 7 sample rows (i) of column j;
# x-weights are per-j (one wide op per chunk), y-weights per-i.
CHUNKS = (OUT,) * OUT
CHMAX = max(CHUNKS)
NIDX = BOX_PER_CORE * NS  # 6272 gathers per tap per core
WCOLS = NIDX // 16        # 392 wrapped index columns

_NC_CACHE = None


def _build_nc():
    """Build + compile the per-core Bass program (same program on all cores)."""
    global _NC_CACHE
    if _NC_CACHE is not None:
        return _NC_CACHE
    from contextlib import ExitStack

    import concourse.bass as bass
    import concourse.tile as tile
    from concourse import bacc, mybir

    fdt = mybir.dt.float32 if FP32 else mybir.dt.float16
    i16 = mybir.dt.int16
    mult = mybir.AluOpType.mult
    add = mybir.AluOpType.add

    nq = int(os.environ.get("KERNEL_NQUEUES", "1"))
    nc = bacc.Bacc(
        "TRN2", target_bir_lowering=False, debug=False, num_devices=NCORES,
        num_swdge_queues=nq,
    )
    # feat_pairs: row r = pixel (y, x) holding [feat[y,x,:], feat[y+1,x,:]]
    feat = nc.dram_tensor("feat", [H * W, 2 * C], fdt, kind="ExternalInput")
    if INDIRECT:
        idx = nc.dram_tensor("idx", [128, NS], mybir.dt.int32, kind="ExternalInput")
    else:
        idx = nc.dram_tensor("idx", [128, WCOLS], i16, kind="ExternalInput")
    wts = nc.dram_tensor("wts", [128, 4 * OUT], mybir.dt.float32, kind="ExternalInput")
    out = nc.dram_tensor("out", [128, NS * C], fdt, kind="ExternalOutput")

    with tile.TileContext(nc) as tc, ExitStack() as ctx:
        meta = ctx.enter_context(tc.tile_pool(name="meta", bufs=1))
        gp = ctx.enter_context(tc.tile_pool(name="g", bufs=4))
        tp = ctx.enter_context(tc.tile_pool(name="t", bufs=4))
        op = ctx.enter_context(tc.tile_pool(name="o", bufs=1))

        if INDIRECT:
            idx_t = meta.tile([128, NS], mybir.dt.int32, name="idx_t")
        else:
            idx_t = meta.tile([128, WCOLS], i16, name="idx_t")
        nc.sync.dma_start(idx_t[:], idx.ap()[:, :])
        wts_t = meta.tile([128, 4 * OUT], mybir.dt.float32, name="wts_t")
        nc.sync.dma_start(wts_t[:], wts.ap()[:, :])

        # Gather source: one elem covers pixels (y,xb),(y,xb+1) with both
        # y/y+1 rows each (row-pair layout), elem_step = one pixel pair.
        if INDIRECT:
            feat_gap = bass.AP(feat, 0, [[2 * C, H * W], [1, 2 * C]])
        else:
            feat_gap = bass.AP(feat, 0, [[2 * C, H * W - 1], [1, 4 * C]])

        # weight table columns: [wx0(j) | wx1(j) | hy(i) | ly(i)] each OUT wide
        WX0, WX1, HY, LY = 0, OUT, 2 * OUT, 3 * OUT
        # y-combine runs in two wide passes over j-groups (y-weights depend
        # only on i, so one op-pair per i spans a whole j-group).
        JG = ((0, 4), (4, 7))
        th = [
            meta.tile([128, hi - lo, OUT, 2, C], fdt, name=f"th_{gi}")
            for gi, (lo, hi) in enumerate(JG)
        ]
        for j in range(OUT):
            # G layout: [128, i(7), xtap(2), ytap(2), C] for column j
            g = gp.tile([128, OUT, 2, 2, C], fdt, tag="g", name=f"g_{j}")
            if INDIRECT:
                nc.gpsimd.indirect_dma_start(
                    out=g[:].rearrange("p k x y c -> p k (x y c)"),
                    out_offset=None,
                    in_=feat_gap,
                    in_offset=bass.IndirectOffsetOnAxis(
                        ap=idx_t[:, j * OUT : (j + 1) * OUT], axis=0
                    ),
                )
            else:
                nc.gpsimd.dma_gather(
                    g[:].rearrange("p k x y c -> p k (x y c)"),
                    feat_gap,
                    idx_t[:, j * OUT * 8 : (j + 1) * OUT * 8],
                    num_idxs=OUT * 128,
                    num_idxs_reg=OUT * 128,
                    elem_size=4 * C,
                    elem_step=2 * C,
                    queue_num=j % nq,
                )
            # x-combine, all 7 rows at once: T[i, ytap, c] = wx0*G[x0] + wx1*G[x1]
            gi = 0 if j < JG[0][1] else 1
            tdst = th[gi][:, j - JG[gi][0]]
            nc.scalar.mul(tdst, g[:, :, 0, :, :], wts_t[:, WX0 + j : WX0 + j + 1])
            nc.vector.scalar_tensor_tensor(
                tdst, g[:, :, 1, :, :], wts_t[:, WX1 + j : WX1 + j + 1],
                tdst, mult, add,
            )
            if j == JG[gi][1] - 1:
                # y-combine for this j-group: O = hy*T[0] + ly*T[1], one
                # wide op-pair per sample row i spanning the whole group.
                lo, hi = JG[gi]
                w = hi - lo
                och = op.tile([128, w, OUT, C], fdt, tag=f"och{gi}",
                              name=f"och_{gi}")
                for i in range(OUT):
                    u = tp.tile([128, w, C], fdt, tag="tmpu", name=f"u_{gi}_{i}")
                    nc.scalar.mul(
                        u[:], th[gi][:, :, i, 1, :], wts_t[:, LY + i : LY + i + 1]
                    )
                    nc.vector.scalar_tensor_tensor(
                        och[:, :, i, :], th[gi][:, :, i, 0, :],
                        wts_t[:, HY + i : HY + i + 1], u[:], mult, add,
                    )
                nc.sync.dma_start(
                    out.ap()[:, lo * OUT * C : hi * OUT * C],
                    och[:].rearrange("p j i c -> p (j i c)"),
                )

    nc.compile()
    _NC_CACHE = nc
    return nc


def _host_tables(boxes):
    """Numpy f32 replica of the reference's index/weight math.

    Returns None if any box is assigned a level other than 4 (never happens
    with the reference's input distribution), else per-core gather tables.
    """
    f32 = np.float32
    b = boxes.astype(f32)
    box_h = b[..., 2] - b[..., 0]
    box_w = b[..., 3] - b[..., 1]
    area = np.sqrt(box_h * box_w)
    with np.errstate(divide="ignore", invalid="ignore"):
        lev = np.floor(np.log(area / f32(224.0)) / np.log(f32(2.0))) + f32(4.0)
    if not np.all(np.isfinite(lev)):
        return None
    levels = np.clip(lev.astype(np.int32), 4, 64)
    if not np.all(levels == 4):
        return None
    scale = np.exp2(levels.astype(f32))
    bs = b / scale[..., None]
    bh = (box_h / scale).astype(f32)
    bw = (box_w / scale).astype(f32)
    by = (bs[..., 0] - f32(0.5)).astype(f32)
    bx = (bs[..., 1] - f32(0.5)).astype(f32)
    offs = ((np.arange(OUT, dtype=f32) + f32(0.5)) / f32(OUT)).astype(f32)
    gy = (by[..., None] + offs * bh[..., None]).astype(f32)  # [B,N,7]
    gx = (bx[..., None] + offs * bw[..., None]).astype(f32)
    y0 = np.maximum(f32(0.0), np.floor(gy))
    x0 = np.maximum(f32(0.0), np.floor(gx))
    bnd = f32(H - 1)
    y_lo = np.minimum(y0, bnd).astype(np.int32)
    y_hi = np.minimum(y0 + f32(1.0), bnd).astype(np.int32)
    x_lo = np.minimum(x0, bnd).astype(np.int32)
    x_hi = np.minimum(x0 + f32(1.0), bnd).astype(np.int32)
    ly = (gy - y0).astype(f32)
    lx = (gx - x0).astype(f32)
    hy = (f32(1.0) - ly).astype(f32)
    hx = (f32(1.0) - lx).astype(f32)
    # 2-pixel gather base in x; remap x-tap weights onto (xb, xb+1)
    xb = np.minimum(x_lo, W - 2)
    wx0 = hx * (x_lo == xb) + lx * (x_hi == xb)
    wx1 = hx * (x_lo == xb + 1) + lx * (x_hi == xb + 1)
    return y_lo, y_hi, xb, hy, ly, wx0.astype(f32), wx1.astype(f32)


def _feat_pairs(feat0_b):
    """[H*W, 2*C] row-pair layout: row (y*W+x) = [feat[y,x,:], feat[y+1,x,:]]
    (last row duplicates y=127, matching the reference's boundary clamp)."""
    fp = np.empty((H, W, 2, C), dtype=FDT)
    fp[:, :, 0] = feat0_b
    fp[:-1, :, 1] = feat0_b[1:]
    fp[-1, :, 1] = feat0_b[-1]
    return np.ascontiguousarray(fp.reshape(H * W, 2 * C))


def _percore_inputs(featp_by_batch, tables, core):
    y_lo, y_hi, xb, hy, ly, wx0, wx1 = tables
    bat, half = divmod(core, 2)
    sl = slice(half * BOX_PER_CORE, (half + 1) * BOX_PER_CORE)
    ylo = y_lo[bat, sl]  # [128, 7]
    xbs = xb[bat, sl]
    # flat pixel index of the 2x2 block base, [128 box, 7 i, 7 j]
    i0 = (ylo[:, :, None] * W + xbs[:, None, :]).astype(np.int32)

    if INDIRECT:
        # natural layout: idx[box, j*7 + i]
        idx = np.transpose(i0, (0, 2, 1)).reshape(128, NS).astype(np.int32)
    else:
        # gather sequence: g = (j*7 + i)*128 + box  (j-major sample order)
        seq = np.transpose(i0, (2, 1, 0)).reshape(NIDX).astype(np.int16)
        wr = seq.reshape(WCOLS, 16).T  # [16, WCOLS]
        idx = np.tile(wr, (8, 1))      # replicate across the 8 gpsimd cores

    hys = hy[bat, sl]    # [128, 7] per sample-row i
    lys = ly[bat, sl]
    wx0s = wx0[bat, sl]  # [128, 7] per sample-col j
    wx1s = wx1[bat, sl]
    wts = np.concatenate([wx0s, wx1s, hys, lys], axis=1).astype(np.float32)

    return {
        "feat": featp_by_batch[bat],
        "idx": np.ascontiguousarray(idx),
        "wts": np.ascontiguousarray(wts),
    }


def _reference_numpy(feats, boxes):
    """Generic fallback: straight numpy port of the reference (never used
    with the reference input distribution; kept for safety)."""
    f32 = np.float32
    L = len(feats)
    padded = np.zeros((B, L, H, W, C), dtype=f32)
    for i, f in enumerate(feats):
        padded[:, i, : f.shape[1], : f.shape[2], :] = f
    b = boxes.astype(f32)
    box_h = b[..., 2] - b[..., 0]
    box_w = b[..., 3] - b[..., 1]
    area = np.sqrt(box_h * box_w)
    lev = np.floor(np.log(area / f32(224.0)) / np.log(f32(2.0))) + f32(4.0)
    levels = np.clip(lev.astype(np.int32), 4, 64)
    scale = np.exp2(levels.astype(f32))
    bs = b / scale[..., None]
    bh = box_h / scale
    bw = box_w / scale
    yxhw = np.concatenate([bs[..., 0:2], bh[..., None], bw[..., None]], axis=-1)
    lvl = levels - 4
    strides = np.exp2(lvl.astype(f32))
    bnd_h = H / strides - f32(1.0)
    bnd_w = W / strides - f32(1.0)
    by = bnd_w[..., None]  # faithful swap from the reference
    bx = bnd_h[..., None]
    box_y = yxhw[..., 0] - f32(0.5)
    box_x = yxhw[..., 1] - f32(0.5)
    offs = (np.arange(OUT, dtype=f32) + f32(0.5)) / f32(OUT)
    gy = box_y[..., None] + offs * yxhw[..., 2:3]
    gx = box_x[..., None] + offs * yxhw[..., 3:4]
    y0 = np.maximum(f32(0.0), np.floor(gy))
    x0 = np.maximum(f32(0.0), np.floor(gx))
    y01 = np.stack([np.minimum(y0, by), np.minimum(y0 + 1, by)], axis=3).reshape(
        B, N, 2 * OUT
    )
    x01 = np.stack([np.minimum(x0, bx), np.minimum(x0 + 1, bx)], axis=3).reshape(
        B, N, 2 * OUT
    )
    yi = y01.astype(np.int32)
    xi = x01.astype(np.int32)
    bi = np.arange(B)[:, None, None, None]
    li = np.clip(lvl, 0, L - 1)[:, :, None, None]
    gathered = padded[bi, li, yi[:, :, :, None], xi[:, :, None, :]]
    ly = gy - y0
    lx = gx - x0
    hy = 1.0 - ly
    hx = 1.0 - lx
    ky = np.stack([hy, ly], axis=3).reshape(B, N, 2 * OUT, 1)
    kx = np.stack([hx, lx], axis=3).reshape(B, N, 1, 2 * OUT)
    kern = (ky * kx * 4.0).astype(f32)
    weighted = gathered * kern[..., None]
    out = weighted.reshape(B, N, OUT, 2, OUT, 2, C).mean(axis=(3, 5))
    return out.astype(f32)


_TRACE_TMPDIR = None


def _run(in_maps, trace=False):
    from concourse.bass_utils import run_bass_kernel_spmd

    nc = _build_nc()
    kw = {}
    if trace and _TRACE_TMPDIR:
        kw["tmpdir"] = _TRACE_TMPDIR
    return run_bass_kernel_spmd(nc, in_maps, list(range(NCORES)), trace=trace, **kw)


def _kernel_impl(inputs, trace=False):
    feats = [np.asarray(inputs[f"feat{i}"], dtype=np.float32) for i in range(5)]
    boxes = np.asarray(inputs["boxes"], dtype=np.float32)
    tables = _host_tables(boxes)
    if tables is None:
        return _reference_numpy(feats, boxes), None
    featp = [_feat_pairs(feats[0][b]) for b in range(B)]
    in_maps = [_percore_inputs(featp, tables, c) for c in range(NCORES)]
    res = _run(in_maps, trace=trace)
    full = np.empty((B, N, OUT, OUT, C), dtype=np.float32)
    for core in range(NCORES):
        bat, half = divmod(core, 2)
        # device sample order is (j, i); transpose back to (i, j)
        o = res.results[core]["out"].astype(np.float32).reshape(
            BOX_PER_CORE, OUT, OUT, C
        ).transpose(0, 2, 1, 3)
        full[bat, half * BOX_PER_CORE : (half + 1) * BOX_PER_CORE] = o
    return full, res


def kernel(**inputs):
    out, _ = _kernel_impl(inputs)
    return out


def kernel_profiled(**inputs):
    """Like kernel() but with trace=True; returns (output, BassKernelResults)."""
    return _kernel_impl(inputs, trace=True)
